# revision 10
# baseline (speedup 1.0000x reference)
"""AdjustableConvolution2d Trainium2 kernel (v6).

Data-parallel over batch: 8 samples -> 8 NeuronCores, no collectives.

The per-sample filter MLP (temp@Wt -> @Wf -> softmax) is tiny, so it runs
on the HOST in fp64/fp32; the device receives finished artifacts:
  - diag: diag(e) stationaries [128, cc*9*128] fp16 (e = unnormalized
    exp(logits), ~1 +- 2%),
  - e32:  the same e as fp32 per-partition scalars for the DVE/ACT taps,
  - wct_s: Wc.T pre-scaled by the softmax denominator 1/sum(e) fp16.
Bias bc is added on the host after the kernel (free), so the device
pipeline is pure dw + 1x1:

  PE (TensorEngine): keepers to warm the HAM clock-gate, then depthwise
    diag-matmuls for cc0 slices 0-7 and cc1 slices 6-7 in slice pairs
    sharing each tap's stationary, interleaved with the 1x1
    channel-combine (2 matmuls/psum per (slice, oc)).
  DVE: cc1 slices 0-5 as three 2-slice slabs in the 66-wide junk-column
    layout; each tap is tensor_scalar (2x mode, ~0.47ns/elem) into a tmp
    then tensor_tensor add (~0.67ns/elem) -- measured 1.9x cheaper than
    the 1x scalar_tensor_tensor fused form.  Some tap scale-steps go to
    the ACT engine (activation scale=e ptr) and some adds to gpsimd to
    keep the DVE off the critical path.
  ACT: PSUM->SBUF mid copies for PE dw units, slab scale-assists, and a
    share of the output copies.  GpSimd: slab add-assists + out copies.
Output fp16 (no bias), upcast + bias on host.
"""

import sys

if "/opt/trn_rl_repo" not in sys.path:
    sys.path.insert(0, "/opt/trn_rl_repo")

import numpy as np

BS, C, H, W = 8, 256, 64, 64
P = 128
CC = C // P            # 2 channel chunks of 128
IW = W + 2             # 66: padded row width (junk-column layout)
IMG = IW * IW          # 4356
RS = 8                 # output rows per slice
NS = RS * W            # 512 columns per slice
NSL = H // RS          # 8 slices
K9 = 9

# blob fp32 column layout (128 partitions)
A_WCT = 0              # wct_s fp16 pairs [p, cc*256+o] -> 256 f32 cols
A_E = 256              # e fp32 [p, cc*9+t] -> 18 cols
A_N = 274

KEEPERS = 135          # junk matmuls warming the PE HAM clock gate
PE1 = (6, 7)           # cc1 slices computed on the PE (rest on DVE slabs)

_CACHE = {}


def _build():
    from contextlib import ExitStack

    import concourse.bass as bass
    import concourse.bacc as bacc
    import concourse.mybir as mybir
    import concourse.tile as tile

    dt = mybir.dt
    f32 = dt.float32
    f16 = dt.float16
    AF = mybir.ActivationFunctionType
    ALU = mybir.AluOpType

    nc = bacc.Bacc(
        "TRN2", target_bir_lowering=False, debug=False, enable_asserts=False
    )

    img_d = nc.dram_tensor("img", [C, IMG], f16, kind="ExternalInput")
    bla_d = nc.dram_tensor("bla", [P, A_N], f32, kind="ExternalInput")
    dia_d = nc.dram_tensor("dia", [P, CC * K9 * P], f16, kind="ExternalInput")
    out_d = nc.dram_tensor("out", [C, H * W], f16, kind="ExternalOutput")

    with tile.TileContext(nc) as tc, ExitStack() as ctx:
        constp = ctx.enter_context(tc.tile_pool(name="const", bufs=1))
        imgp = ctx.enter_context(tc.tile_pool(name="img", bufs=1))
        accp = ctx.enter_context(tc.tile_pool(name="accp", bufs=1))
        tmpp = ctx.enter_context(tc.tile_pool(name="tmpp", bufs=4))
        midsb = ctx.enter_context(tc.tile_pool(name="midsb", bufs=12))
        outsb = ctx.enter_context(tc.tile_pool(name="outsb", bufs=6))
        midps = ctx.enter_context(
            tc.tile_pool(name="midps", bufs=3, space=bass.MemorySpace.PSUM)
        )
        outps = ctx.enter_context(
            tc.tile_pool(name="outps", bufs=3, space=bass.MemorySpace.PSUM)
        )

        # tiny fp16 zero tile for PE keepers, available immediately
        kzero = constp.tile([P, 32], f16)
        nc.gpsimd.memset(kzero[:], 0.0)

        # ---- input DMAs, priority order: the PE's first dw pair needs dia
        # cc0 + img cc0 rows 0..9; the first DVE slab needs e32 (bla) + img
        # cc1 rows 0..17.  ~3MB of input is bandwidth-bound (~8us), so the
        # first-needed pieces go first across both queues.
        bla = constp.tile([P, A_N], f32)
        dia = constp.tile([P, CC, K9, P], f16)
        dia_v = dia_d.rearrange("p (cc t q) -> p cc t q", cc=CC, t=K9)
        img_sb = imgp.tile([P, CC, IMG], f16)

        nc.sync.dma_start(dia[:, 0], dia_v[:, 0])
        nc.scalar.dma_start(img_sb[:, 1, : 18 * IW], img_d[P : 2 * P, : 18 * IW])
        nc.sync.dma_start(img_sb[:, 0, : 10 * IW], img_d[0:P, : 10 * IW])
        nc.scalar.dma_start(bla[:], bla_d[:])
        nc.sync.dma_start(
            img_sb[:, 0, 10 * IW : 18 * IW], img_d[0:P, 10 * IW : 18 * IW]
        )
        nc.sync.dma_start(dia[:, 1], dia_v[:, 1])
        splits = ((18 * IW, 36 * IW), (36 * IW, 52 * IW), (52 * IW, IMG))
        for lo, hi in splits:
            nc.sync.dma_start(img_sb[:, 0, lo:hi], img_d[0:P, lo:hi])
            nc.scalar.dma_start(img_sb[:, 1, lo:hi], img_d[P : 2 * P, lo:hi])

        wct_v = bla[:, A_WCT : A_WCT + 256].bitcast(f16).rearrange(
            "p (cc o) -> p cc o", cc=CC
        )
        e32 = bla[:, A_E : A_E + CC * K9].rearrange("p (cc t) -> p cc t", cc=CC)

        # ---- PE keepers: warm the HAM clock gate while DMAs land
        k_ps = midps.tile([P, NS], f32, name="mid", tag="mid")
        for _ in range(KEEPERS):
            nc.tensor.matmul(k_ps[:1, :32], kzero[:, :1], kzero[:], skip_group_check=True)

        imgv = [
            img_sb[:, cc, :].rearrange("p (r w) -> p r w", w=IW) for cc in range(CC)
        ]
        img1 = img_sb[:, 1, :]

        # ---- DVE slabs: cc1 slices 0-5 as three 2-slice slabs.
        # Per tap: tmp = img_view * e (TS on DVE at 2x, or ACT activation),
        # then acc += tmp (TT on DVE at 2x).  Two taps are scaled on DVE
        # but SUMMED off-chain on gpsimd (g = tmp7 + tmp8) so the slow
        # gpsimd op overlaps the DVE chain instead of extending it.
        ACT_TAPS = (1, 4, 7)       # scale-step on ACT
        GP_TAPS = (5, 8)           # summed off-chain on gpsimd

        def dve_slab(y0, nrows):
            fd = (nrows - 1) * IW + W
            acc = accp.tile([P, nrows * IW], f16, name=f"acc{y0}", tag=f"acc{y0}")
            sc = lambda t: e32[:, 1, t : t + 1]

            def view(t9):
                di, dj = t9 // 3, t9 % 3
                s0 = (y0 + di) * IW + dj
                return img1[:, s0 : s0 + fd]

            tmps = {}
            # scale-steps for ACT and gpsimd taps first, so they run ahead
            for t9 in ACT_TAPS + GP_TAPS:
                tm = tmpp.tile([P, fd], f16, name=f"tm{y0}_{t9}", tag=f"tmp{t9}")
                if t9 in ACT_TAPS:
                    nc.scalar.activation(tm[:], view(t9), AF.Identity, scale=sc(t9))
                else:
                    nc.vector.tensor_scalar_mul(tm[:], view(t9), sc(t9))
                tmps[t9] = tm
            gsum = tmpp.tile([P, fd], f16, name=f"gs{y0}", tag="gsum")
            nc.gpsimd.tensor_tensor(
                gsum[:], tmps[GP_TAPS[0]][:], tmps[GP_TAPS[1]][:], op=ALU.add
            )
            # DVE chain: TS into acc, then TT adds; gsum folded at the end
            nc.vector.tensor_scalar_mul(acc[:, :fd], view(0), sc(0))
            for i, t9 in enumerate((1, 4, 7, 2, 3, 6)):
                if t9 in tmps:
                    tm = tmps[t9]
                else:
                    tm = tmpp.tile([P, fd], f16, name=f"tm{y0}_{t9}", tag=f"tmd{i % 2}")
                    nc.vector.tensor_scalar_mul(tm[:], view(t9), sc(t9))
                nc.vector.tensor_tensor(acc[:, :fd], tm[:], acc[:, :fd], op=ALU.add)
            nc.vector.tensor_tensor(acc[:, :fd], gsum[:], acc[:, :fd], op=ALU.add)
            return acc.rearrange("p (r w) -> p r w", w=IW)

        slabs = {}

        def mid1(hs):
            if hs in PE1:
                return None  # comes from PE mid copies
            slab = slabs[hs // 2]
            return slab[:, RS * (hs % 2) : RS * (hs % 2) + RS, :W]

        m0, m1 = {}, {}

        # ---- PE depthwise in slice pairs sharing each tap's stationary
        def dw_pair(cc, hs_a, hs_b=None):
            mts = []
            for hs in (hs_a, hs_b):
                if hs is None:
                    continue
                mts.append((hs, midps.tile([P, NS], f32, name="mid", tag="mid")))
            for t9 in range(K9):
                di, dj = t9 // 3, t9 % 3
                for hs, mt in mts:
                    nc.tensor.matmul(
                        mt[:],
                        dia[:, cc, t9, :],
                        imgv[cc][:, RS * hs + di : RS * hs + di + RS, dj : dj + W],
                        start=(t9 == 0),
                        stop=(t9 == 8),
                    )
            for hs, mt in mts:
                m = midsb.tile([P, NS], f16, name="m", tag="m")
                nc.scalar.copy(m[:], mt[:])
                (m0 if cc == 0 else m1)[hs] = m

        outv = out_d.rearrange("(o p) hw -> p o hw", o=CC)

        # out-copy engine rotation: ACT carries slab assists, DVE is busy
        # with slabs until ~2/3 in, gpsimd is idle -> early slices lean on
        # ACT+gpsimd, late slices on DVE.
        def out_copy(ob, oc, o_ps, eng):
            if eng == "v":
                nc.vector.tensor_copy(ob[:, oc, :], o_ps[:])
            else:
                nc.scalar.copy(ob[:, oc, :], o_ps[:])

        def one_by_one(hs, engs=("s", "s"), last=False):
            rhs0 = m0[hs][:]
            rhs1 = m1[hs][:] if hs in PE1 else mid1(hs)
            ob = outsb.tile([P, CC, NS], f16, name="ob", tag="ob")
            for oc in range(CC):
                o_ps = outps.tile([P, NS], f32, name="ops", tag="ops")
                nc.tensor.matmul(
                    o_ps[:], wct_v[:, 0, oc * P : (oc + 1) * P], rhs0,
                    start=True, stop=False,
                )
                nc.tensor.matmul(
                    o_ps[:], wct_v[:, 1, oc * P : (oc + 1) * P], rhs1,
                    start=False, stop=True,
                )
                out_copy(ob, oc, o_ps, engs[oc])
                if last:
                    dst = outv[:, :, hs * NS : (hs + 1) * NS]
                    eng = nc.sync if oc == 0 else nc.scalar
                    eng.dma_start(dst[:, oc : oc + 1, :], ob[:, oc : oc + 1, :])
            if not last:
                dst = outv[:, :, hs * NS : (hs + 1) * NS]
                nc.sync.dma_start(dst[:, :, :], ob[:, :, :])

        # ---- emission order (Tile scheduler refines by deps)
        slabs[0] = dve_slab(0, 16)
        dw_pair(0, 0, 1)
        dw_pair(0, 2, 3)
        slabs[1] = dve_slab(16, 16)
        one_by_one(0, engs=("s", "s"))
        one_by_one(1, engs=("s", "s"))
        dw_pair(0, 4, 5)
        slabs[2] = dve_slab(32, 16)
        one_by_one(2, engs=("s", "s"))
        one_by_one(3, engs=("s", "s"))
        dw_pair(0, 6, 7)
        dw_pair(1, PE1[0], PE1[1])
        one_by_one(4, engs=("v", "s"))
        one_by_one(5, engs=("s", "v"))
        one_by_one(6, engs=("v", "s"))
        one_by_one(7, engs=("v", "s"), last=True)

    nc.compile()
    return nc


def _get_nc():
    if "nc" not in _CACHE:
        _CACHE["nc"] = _build()
    return _CACHE["nc"]


def _prep_in_maps(image_feat, temp_feat, Wt, bt, Wf, bf, Wc, bc):
    f = lambda a: np.asarray(a, dtype=np.float64)
    image_feat = np.asarray(image_feat, dtype=np.float32)

    # ---- host MLP: per-sample unnormalized softmax weights
    t = f(temp_feat) @ f(Wt) + f(bt)                      # [bs, 32]
    logit = (t @ f(Wf) + f(bf)) / 100.0                   # [bs, c*9]
    e = np.exp(logit).reshape(BS, C, K9)                  # unnormalized
    r = 1.0 / e.sum(-1)                                   # [bs, c]

    img_pad = np.zeros((BS, C, IW, IW), np.float16)
    img_pad[:, :, 1 : H + 1, 1 : W + 1] = image_feat.astype(np.float16)
    img_pad = img_pad.reshape(BS, C, IMG)

    wct = f(Wc).T                                         # [c, o]

    in_maps = []
    for i in range(BS):
        blob = np.zeros((P, A_N), np.float32)
        wct_s = (wct * r[i][:, None]).astype(np.float16)  # [c, o]
        wct_p = np.ascontiguousarray(
            wct_s.reshape(CC, P, C).transpose(1, 0, 2).reshape(P, CC * C)
        )
        blob[:, A_WCT : A_WCT + 256] = wct_p.view(np.float32)
        # e fp32 scalars [p, cc*9+t]
        ei = e[i].reshape(CC, P, K9).transpose(1, 0, 2).reshape(P, CC * K9)
        blob[:, A_E : A_E + CC * K9] = ei.astype(np.float32)
        # diag stationaries [p, cc, t, q] fp16: diag(e)
        dia = np.zeros((P, CC, K9, P), np.float16)
        idx = np.arange(P)
        for cc in range(CC):
            for t9 in range(K9):
                dia[idx, cc, t9, idx] = e[i, cc * P : (cc + 1) * P, t9].astype(
                    np.float16
                )
        in_maps.append(
            {
                "img": img_pad[i],
                "bla": blob,
                "dia": dia.reshape(P, CC * K9 * P),
            }
        )
    return in_maps


def kernel(image_feat, temp_feat, Wt, bt, Wf, bf, Wc, bc):
    from concourse.bass_utils import run_bass_kernel_spmd

    nc = _get_nc()
    in_maps = _prep_in_maps(image_feat, temp_feat, Wt, bt, Wf, bf, Wc, bc)
    res = run_bass_kernel_spmd(nc, in_maps, core_ids=list(range(BS)))
    _CACHE["last_result"] = res
    out = np.stack([res.results[i]["out"] for i in range(BS)], axis=0)
    out = out.reshape(BS, C, H, W).astype(np.float32)
    out += np.asarray(bc, dtype=np.float32)[None, :, None, None]
    return out


# revision 13
# speedup vs baseline: 1.1527x; 1.1527x over previous
"""AdjustableConvolution2d Trainium2 kernel (v6).

Data-parallel over batch: 8 samples -> 8 NeuronCores, no collectives.

The per-sample filter MLP (temp@Wt -> @Wf -> softmax) is tiny, so it runs
on the HOST in fp64/fp32; the device receives finished artifacts:
  - diag: diag(e) stationaries [128, cc*9*128] fp16 (e = unnormalized
    exp(logits), ~1 +- 2%),
  - e32:  the same e as fp32 per-partition scalars for the DVE/ACT taps,
  - wct_s: Wc.T pre-scaled by the softmax denominator 1/sum(e) fp16.
Bias bc is added on the host after the kernel (free), so the device
pipeline is pure dw + 1x1:

  PE (TensorEngine): keepers to warm the HAM clock-gate, then depthwise
    diag-matmuls for cc0 slices 0-7 and cc1 slices 6-7 in slice pairs
    sharing each tap's stationary, interleaved with the 1x1
    channel-combine (2 matmuls/psum per (slice, oc)).
  DVE: cc1 slices 0-5 as three 2-slice slabs in the 66-wide junk-column
    layout; each tap is tensor_scalar (2x mode, ~0.47ns/elem) into a tmp
    then tensor_tensor add (~0.67ns/elem) -- measured 1.9x cheaper than
    the 1x scalar_tensor_tensor fused form.  Some tap scale-steps go to
    the ACT engine (activation scale=e ptr) and some adds to gpsimd to
    keep the DVE off the critical path.
  ACT: PSUM->SBUF mid copies for PE dw units, slab scale-assists, and a
    share of the output copies.  GpSimd: slab add-assists + out copies.
Output fp16 (no bias), upcast + bias on host.
"""

import sys

if "/opt/trn_rl_repo" not in sys.path:
    sys.path.insert(0, "/opt/trn_rl_repo")

import numpy as np

BS, C, H, W = 8, 256, 64, 64
P = 128
CC = C // P            # 2 channel chunks of 128
IW = W + 2             # 66: padded row width (junk-column layout)
IMG = IW * IW          # 4356
RS = 8                 # output rows per slice
NS = RS * W            # 512 columns per slice
NSL = H // RS          # 8 slices
K9 = 9

# blob fp32 column layout (128 partitions)
A_WCT = 0              # wct_s fp16 pairs [p, cc*256+o] -> 256 f32 cols
A_E = 256              # e fp32 [p, cc*9+t] -> 18 cols
A_N = 274

KEEPERS = 135          # junk matmuls warming the PE HAM clock gate
PE1 = (6, 7)           # cc1 slices computed on the PE (rest on DVE slabs)

_CACHE = {}


def _build():
    from contextlib import ExitStack

    import concourse.bass as bass
    import concourse.bacc as bacc
    import concourse.mybir as mybir
    import concourse.tile as tile

    dt = mybir.dt
    f32 = dt.float32
    f16 = dt.float16
    AF = mybir.ActivationFunctionType
    ALU = mybir.AluOpType

    nc = bacc.Bacc(
        "TRN2", target_bir_lowering=False, debug=False, enable_asserts=False
    )

    img_d = nc.dram_tensor("img", [C, IMG], f16, kind="ExternalInput")
    bla_d = nc.dram_tensor("bla", [P, A_N], f32, kind="ExternalInput")
    dia_d = nc.dram_tensor("dia", [P, CC * K9 * P], f16, kind="ExternalInput")
    out_d = nc.dram_tensor("out", [C, H * W], f16, kind="ExternalOutput")

    with tile.TileContext(nc) as tc, ExitStack() as ctx:
        constp = ctx.enter_context(tc.tile_pool(name="const", bufs=1))
        imgp = ctx.enter_context(tc.tile_pool(name="img", bufs=1))
        accp = ctx.enter_context(tc.tile_pool(name="accp", bufs=1))
        tmpp = ctx.enter_context(tc.tile_pool(name="tmpp", bufs=1))
        midsb = ctx.enter_context(tc.tile_pool(name="midsb", bufs=12))
        outsb = ctx.enter_context(tc.tile_pool(name="outsb", bufs=6))
        midps = ctx.enter_context(
            tc.tile_pool(name="midps", bufs=3, space=bass.MemorySpace.PSUM)
        )
        outps = ctx.enter_context(
            tc.tile_pool(name="outps", bufs=3, space=bass.MemorySpace.PSUM)
        )

        # tiny fp16 zero tile for PE keepers, available immediately
        kzero = constp.tile([P, 32], f16)
        nc.gpsimd.memset(kzero[:], 0.0)

        # ---- input DMAs, priority order: the PE's first dw pair needs dia
        # cc0 + img cc0 rows 0..9; the first DVE slab needs e32 (bla) + img
        # cc1 rows 0..17.  ~3MB of input is bandwidth-bound (~8us), so the
        # first-needed pieces go first across both queues.
        bla = constp.tile([P, A_N], f32)
        dia = constp.tile([P, CC, K9, P], f16)
        dia_v = dia_d.rearrange("p (cc t q) -> p cc t q", cc=CC, t=K9)
        img_sb = imgp.tile([P, CC, IMG], f16)

        nc.sync.dma_start(dia[:, 0], dia_v[:, 0])
        nc.scalar.dma_start(img_sb[:, 1, : 18 * IW], img_d[P : 2 * P, : 18 * IW])
        nc.sync.dma_start(img_sb[:, 0, : 10 * IW], img_d[0:P, : 10 * IW])
        nc.scalar.dma_start(bla[:], bla_d[:])
        nc.sync.dma_start(
            img_sb[:, 0, 10 * IW : 18 * IW], img_d[0:P, 10 * IW : 18 * IW]
        )
        nc.sync.dma_start(dia[:, 1], dia_v[:, 1])
        splits = ((18 * IW, 36 * IW), (36 * IW, 52 * IW), (52 * IW, IMG))
        for lo, hi in splits:
            nc.sync.dma_start(img_sb[:, 0, lo:hi], img_d[0:P, lo:hi])
            nc.scalar.dma_start(img_sb[:, 1, lo:hi], img_d[P : 2 * P, lo:hi])

        wct_v = bla[:, A_WCT : A_WCT + 256].bitcast(f16).rearrange(
            "p (cc o) -> p cc o", cc=CC
        )
        e32 = bla[:, A_E : A_E + CC * K9].rearrange("p (cc t) -> p cc t", cc=CC)

        # ---- PE keepers: warm the HAM clock gate while DMAs land
        k_ps = midps.tile([P, NS], f32, name="mid", tag="mid")
        for _ in range(KEEPERS):
            nc.tensor.matmul(k_ps[:1, :32], kzero[:, :1], kzero[:], skip_group_check=True)

        imgv = [
            img_sb[:, cc, :].rearrange("p (r w) -> p r w", w=IW) for cc in range(CC)
        ]
        img1 = img_sb[:, 1, :]

        # ---- DVE slabs: cc1 slices 0-5 as three 2-slice slabs.
        # Per tap: tmp = img_view * e (TS on DVE at 2x, or ACT activation),
        # then acc += tmp (TT on DVE at 2x).  Four taps pair-sum off-chain
        # on gpsimd (g1 = t2+t5, g2 = t6+t8) so the slow gpsimd ops overlap
        # the DVE chain.  Chain order: own-scaled tap first, ACT/gp-fed adds
        # last, so cross-engine results have time to land.
        ACT_TAPS = (1, 4, 7)       # scale-step on ACT
        GP_PAIRS = ((2, 5), (6, 8))  # scaled on DVE, summed on gpsimd

        def dve_slab(y0, nrows):
            fd = (nrows - 1) * IW + W
            acc = accp.tile([P, nrows * IW], f16, name=f"acc{y0}", tag=f"acc{y0}")
            sc = lambda t: e32[:, 1, t : t + 1]

            def view(t9):
                di, dj = t9 // 3, t9 % 3
                s0 = (y0 + di) * IW + dj
                return img1[:, s0 : s0 + fd]

            tmps = {}
            # gp-tap scale-steps first (DVE), then ACT scale-steps (scalar)
            for pa, pb in GP_PAIRS:
                for t9 in (pa, pb):
                    tm = tmpp.tile([P, fd], f16, name=f"tm{y0}_{t9}", tag=f"tmp{t9}")
                    nc.vector.tensor_scalar_mul(tm[:], view(t9), sc(t9))
                    tmps[t9] = tm
            for t9 in ACT_TAPS:
                tm = tmpp.tile([P, fd], f16, name=f"tm{y0}_{t9}", tag=f"tmp{t9}")
                nc.scalar.activation(tm[:], view(t9), AF.Identity, scale=sc(t9))
                tmps[t9] = tm
            gs = []
            for gi, (pa, pb) in enumerate(GP_PAIRS):
                g = tmpp.tile([P, fd], f16, name=f"gs{y0}_{gi}", tag=f"gsum{gi}")
                nc.gpsimd.tensor_tensor(g[:], tmps[pa][:], tmps[pb][:], op=ALU.add)
                gs.append(g)
            # DVE chain
            nc.vector.tensor_scalar_mul(acc[:, :fd], view(0), sc(0))
            tm3 = tmpp.tile([P, fd], f16, name=f"tm{y0}_3", tag="tmd0")
            nc.vector.tensor_scalar_mul(tm3[:], view(3), sc(3))
            for tm in (tm3, tmps[1], tmps[4], tmps[7], gs[0], gs[1]):
                nc.vector.tensor_tensor(acc[:, :fd], tm[:], acc[:, :fd], op=ALU.add)
            return acc.rearrange("p (r w) -> p r w", w=IW)

        slabs = {}

        def mid1(hs):
            if hs in PE1:
                return None  # comes from PE mid copies
            slab = slabs[hs // 2]
            return slab[:, RS * (hs % 2) : RS * (hs % 2) + RS, :W]

        m0, m1 = {}, {}

        # ---- PE depthwise in slice pairs sharing each tap's stationary
        def dw_pair(cc, hs_a, hs_b=None):
            mts = []
            for hs in (hs_a, hs_b):
                if hs is None:
                    continue
                mts.append((hs, midps.tile([P, NS], f32, name="mid", tag="mid")))
            for t9 in range(K9):
                di, dj = t9 // 3, t9 % 3
                for hs, mt in mts:
                    nc.tensor.matmul(
                        mt[:],
                        dia[:, cc, t9, :],
                        imgv[cc][:, RS * hs + di : RS * hs + di + RS, dj : dj + W],
                        start=(t9 == 0),
                        stop=(t9 == 8),
                    )
            for hs, mt in mts:
                m = midsb.tile([P, NS], f16, name="m", tag="m")
                nc.scalar.copy(m[:], mt[:])
                (m0 if cc == 0 else m1)[hs] = m

        outv = out_d.rearrange("(o p) hw -> p o hw", o=CC)

        # out-copy engine rotation: ACT carries slab assists, DVE is busy
        # with slabs until ~2/3 in, gpsimd is idle -> early slices lean on
        # ACT+gpsimd, late slices on DVE.
        def out_copy(ob, oc, o_ps, eng):
            if eng == "v":
                nc.vector.tensor_copy(ob[:, oc, :], o_ps[:])
            else:
                nc.scalar.copy(ob[:, oc, :], o_ps[:])

        def one_by_one(hs, engs=("s", "s"), last=False):
            rhs0 = m0[hs][:]
            rhs1 = m1[hs][:] if hs in PE1 else mid1(hs)
            ob = outsb.tile([P, CC, NS], f16, name="ob", tag="ob")
            for oc in range(CC):
                o_ps = outps.tile([P, NS], f32, name="ops", tag="ops")
                nc.tensor.matmul(
                    o_ps[:], wct_v[:, 0, oc * P : (oc + 1) * P], rhs0,
                    start=True, stop=False,
                )
                nc.tensor.matmul(
                    o_ps[:], wct_v[:, 1, oc * P : (oc + 1) * P], rhs1,
                    start=False, stop=True,
                )
                out_copy(ob, oc, o_ps, engs[oc])
                if last:
                    dst = outv[:, :, hs * NS : (hs + 1) * NS]
                    eng = nc.sync if oc == 0 else nc.scalar
                    eng.dma_start(dst[:, oc : oc + 1, :], ob[:, oc : oc + 1, :])
            if not last:
                dst = outv[:, :, hs * NS : (hs + 1) * NS]
                nc.sync.dma_start(dst[:, :, :], ob[:, :, :])

        # ---- emission order (Tile scheduler refines by deps)
        slabs[0] = dve_slab(0, 16)
        dw_pair(0, 0, 1)
        dw_pair(0, 2, 3)
        one_by_one(0, engs=("s", "s"))
        one_by_one(1, engs=("s", "s"))
        slabs[1] = dve_slab(16, 16)
        dw_pair(0, 4, 5)
        one_by_one(2, engs=("s", "s"))
        one_by_one(3, engs=("s", "s"))
        slabs[2] = dve_slab(32, 16)
        dw_pair(0, 6, 7)
        dw_pair(1, PE1[0], PE1[1])
        one_by_one(4, engs=("v", "s"))
        one_by_one(5, engs=("s", "v"))
        one_by_one(6, engs=("v", "s"))
        one_by_one(7, engs=("v", "s"), last=True)

    nc.compile()
    return nc


def _get_nc():
    if "nc" not in _CACHE:
        _CACHE["nc"] = _build()
    return _CACHE["nc"]


def _prep_in_maps(image_feat, temp_feat, Wt, bt, Wf, bf, Wc, bc):
    f = lambda a: np.asarray(a, dtype=np.float64)
    image_feat = np.asarray(image_feat, dtype=np.float32)

    # ---- host MLP: per-sample unnormalized softmax weights
    t = f(temp_feat) @ f(Wt) + f(bt)                      # [bs, 32]
    logit = (t @ f(Wf) + f(bf)) / 100.0                   # [bs, c*9]
    e = np.exp(logit).reshape(BS, C, K9)                  # unnormalized
    r = 1.0 / e.sum(-1)                                   # [bs, c]

    img_pad = np.zeros((BS, C, IW, IW), np.float16)
    img_pad[:, :, 1 : H + 1, 1 : W + 1] = image_feat.astype(np.float16)
    img_pad = img_pad.reshape(BS, C, IMG)

    wct = f(Wc).T                                         # [c, o]

    in_maps = []
    for i in range(BS):
        blob = np.zeros((P, A_N), np.float32)
        wct_s = (wct * r[i][:, None]).astype(np.float16)  # [c, o]
        wct_p = np.ascontiguousarray(
            wct_s.reshape(CC, P, C).transpose(1, 0, 2).reshape(P, CC * C)
        )
        blob[:, A_WCT : A_WCT + 256] = wct_p.view(np.float32)
        # e fp32 scalars [p, cc*9+t]
        ei = e[i].reshape(CC, P, K9).transpose(1, 0, 2).reshape(P, CC * K9)
        blob[:, A_E : A_E + CC * K9] = ei.astype(np.float32)
        # diag stationaries [p, cc, t, q] fp16: diag(e)
        dia = np.zeros((P, CC, K9, P), np.float16)
        idx = np.arange(P)
        for cc in range(CC):
            for t9 in range(K9):
                dia[idx, cc, t9, idx] = e[i, cc * P : (cc + 1) * P, t9].astype(
                    np.float16
                )
        in_maps.append(
            {
                "img": img_pad[i],
                "bla": blob,
                "dia": dia.reshape(P, CC * K9 * P),
            }
        )
    return in_maps


def kernel(image_feat, temp_feat, Wt, bt, Wf, bf, Wc, bc):
    from concourse.bass_utils import run_bass_kernel_spmd

    nc = _get_nc()
    in_maps = _prep_in_maps(image_feat, temp_feat, Wt, bt, Wf, bf, Wc, bc)
    res = run_bass_kernel_spmd(nc, in_maps, core_ids=list(range(BS)))
    _CACHE["last_result"] = res
    out = np.stack([res.results[i]["out"] for i in range(BS)], axis=0)
    out = out.reshape(BS, C, H, W).astype(np.float32)
    out += np.asarray(bc, dtype=np.float32)[None, :, None, None]
    return out


# revision 17
# speedup vs baseline: 1.3294x; 1.1533x over previous
"""AdjustableConvolution2d Trainium2 kernel (v6).

Data-parallel over batch: 8 samples -> 8 NeuronCores, no collectives.

The per-sample filter MLP (temp@Wt -> @Wf -> softmax) is tiny, so it runs
on the HOST in fp64/fp32; the device receives finished artifacts:
  - diag: diag(e) stationaries [128, cc*9*128] fp16 (e = unnormalized
    exp(logits), ~1 +- 2%),
  - e32:  the same e as fp32 per-partition scalars for the DVE/ACT taps,
  - wct_s: Wc.T pre-scaled by the softmax denominator 1/sum(e) fp16.
Bias bc is added on the host after the kernel (free), so the device
pipeline is pure dw + 1x1:

  PE (TensorEngine): keepers to warm the HAM clock-gate, then depthwise
    diag-matmuls for cc0 slices 0-7 and cc1 slices 6-7 in slice pairs
    sharing each tap's stationary, interleaved with the 1x1
    channel-combine (2 matmuls/psum per (slice, oc)).
  DVE: cc1 slices 0-5 as three 2-slice slabs in the 66-wide junk-column
    layout; each tap is tensor_scalar (2x mode, ~0.47ns/elem) into a tmp
    then tensor_tensor add (~0.67ns/elem) -- measured 1.9x cheaper than
    the 1x scalar_tensor_tensor fused form.  Some tap scale-steps go to
    the ACT engine (activation scale=e ptr) and some adds to gpsimd to
    keep the DVE off the critical path.
  ACT: PSUM->SBUF mid copies for PE dw units, slab scale-assists, and a
    share of the output copies.  GpSimd: slab add-assists + out copies.
Output fp16 (no bias), upcast + bias on host.
"""

import sys

if "/opt/trn_rl_repo" not in sys.path:
    sys.path.insert(0, "/opt/trn_rl_repo")

import numpy as np

BS, C, H, W = 8, 256, 64, 64
P = 128
CC = C // P            # 2 channel chunks of 128
IW = W + 2             # 66: padded row width (junk-column layout)
IMG = IW * IW          # 4356
RS = 8                 # output rows per slice
NS = RS * W            # 512 columns per slice
NSL = H // RS          # 8 slices
K9 = 9

# blob fp32 column layout (128 partitions)
A_WCT = 0              # wct_s fp16 pairs [p, cc*256+o] -> 256 f32 cols
A_E = 256              # e fp32 [p, cc*9+t] -> 18 cols
A_N = 274

KEEPERS = 135          # junk matmuls warming the PE HAM clock gate
PE1 = (5, 6, 7)        # cc1 slices computed on the PE (rest on DVE slabs)

_CACHE = {}


def _build():
    from contextlib import ExitStack

    import concourse.bass as bass
    import concourse.bacc as bacc
    import concourse.mybir as mybir
    import concourse.tile as tile

    dt = mybir.dt
    f32 = dt.float32
    f16 = dt.float16
    AF = mybir.ActivationFunctionType
    ALU = mybir.AluOpType

    nc = bacc.Bacc(
        "TRN2", target_bir_lowering=False, debug=False, enable_asserts=False
    )

    img_d = nc.dram_tensor("img", [C, IMG], f16, kind="ExternalInput")
    bla_d = nc.dram_tensor("bla", [P, A_N], f32, kind="ExternalInput")
    dia_d = nc.dram_tensor("dia", [P, CC * K9 * P], f16, kind="ExternalInput")
    out_d = nc.dram_tensor("out", [C, H * W], f16, kind="ExternalOutput")

    with tile.TileContext(nc) as tc, ExitStack() as ctx:
        constp = ctx.enter_context(tc.tile_pool(name="const", bufs=1))
        imgp = ctx.enter_context(tc.tile_pool(name="img", bufs=1))
        accp = ctx.enter_context(tc.tile_pool(name="accp", bufs=1))
        tmpp = ctx.enter_context(tc.tile_pool(name="tmpp", bufs=1))
        midsb = ctx.enter_context(tc.tile_pool(name="midsb", bufs=12))
        outsb = ctx.enter_context(tc.tile_pool(name="outsb", bufs=6))
        midps = ctx.enter_context(
            tc.tile_pool(name="midps", bufs=3, space=bass.MemorySpace.PSUM)
        )
        outps = ctx.enter_context(
            tc.tile_pool(name="outps", bufs=3, space=bass.MemorySpace.PSUM)
        )

        # tiny fp16 zero tile for PE keepers, available immediately
        kzero = constp.tile([P, 32], f16)
        nc.gpsimd.memset(kzero[:], 0.0)

        # ---- input DMAs, priority order: the PE's first dw pair needs dia
        # cc0 + img cc0 rows 0..9; the first DVE slab needs e32 (bla) + img
        # cc1 rows 0..17.  ~3MB of input is bandwidth-bound (~8us), so the
        # first-needed pieces go first across both queues.
        bla = constp.tile([P, A_N], f32)
        dia = constp.tile([P, CC, K9, P], f16)
        dia_v = dia_d.rearrange("p (cc t q) -> p cc t q", cc=CC, t=K9)
        img_sb = imgp.tile([P, CC, IMG], f16)

        # all input DMAs ride the sync queue (the scalar/ACT queue must stay
        # clear for slab assists + copies); priority order by first need.
        nc.sync.dma_start(dia[:, 0], dia_v[:, 0])
        nc.sync.dma_start(img_sb[:, 0, : 10 * IW], img_d[0:P, : 10 * IW])
        nc.sync.dma_start(img_sb[:, 1, : 18 * IW], img_d[P : 2 * P, : 18 * IW])
        nc.sync.dma_start(bla[:], bla_d[:])
        nc.sync.dma_start(
            img_sb[:, 0, 10 * IW : 18 * IW], img_d[0:P, 10 * IW : 18 * IW]
        )
        nc.sync.dma_start(dia[:, 1], dia_v[:, 1])
        splits = ((18 * IW, 36 * IW), (36 * IW, 52 * IW), (52 * IW, IMG))
        for lo, hi in splits:
            nc.sync.dma_start(img_sb[:, 0, lo:hi], img_d[0:P, lo:hi])
            nc.sync.dma_start(img_sb[:, 1, lo:hi], img_d[P : 2 * P, lo:hi])

        wct_v = bla[:, A_WCT : A_WCT + 256].bitcast(f16).rearrange(
            "p (cc o) -> p cc o", cc=CC
        )
        e32 = bla[:, A_E : A_E + CC * K9].rearrange("p (cc t) -> p cc t", cc=CC)

        # ---- PE keepers: warm the HAM clock gate while DMAs land
        k_ps = midps.tile([P, NS], f32, name="mid", tag="mid")
        for _ in range(KEEPERS):
            nc.tensor.matmul(k_ps[:1, :32], kzero[:, :1], kzero[:], skip_group_check=True)

        imgv = [
            img_sb[:, cc, :].rearrange("p (r w) -> p r w", w=IW) for cc in range(CC)
        ]
        img1 = img_sb[:, 1, :]

        # ---- DVE slabs: cc1 slices 0-4 as two 2-slice slabs + one single.
        # Per tap: tmp = img_view * e (TS on DVE at ~2x, or ACT activation),
        # then acc += tmp (TT on DVE at ~2x).  No gpsimd: its tensor ops
        # lock the DVE's shared SBUF port and throttle concurrent DVE work
        # ~5x (measured).  Chain order: own-scaled taps first, ACT-fed adds
        # last, so the cross-engine tmps have time to land.
        ACT_TAPS = (1, 4, 7)       # scale-step on ACT

        def dve_slab(y0, nrows):
            fd = (nrows - 1) * IW + W
            acc = accp.tile([P, nrows * IW], f16, name=f"acc{y0}", tag=f"acc{y0}")
            sc = lambda t: e32[:, 1, t : t + 1]

            def view(t9):
                di, dj = t9 // 3, t9 % 3
                s0 = (y0 + di) * IW + dj
                return img1[:, s0 : s0 + fd]

            tmps = {}
            for t9 in ACT_TAPS:
                tm = tmpp.tile([P, fd], f16, name=f"tm{y0}_{t9}", tag=f"tmp{t9}")
                nc.scalar.activation(tm[:], view(t9), AF.Identity, scale=sc(t9))
                tmps[t9] = tm
            # DVE chain: own taps (TS+TT) first, ACT-fed adds last
            nc.vector.tensor_scalar_mul(acc[:, :fd], view(0), sc(0))
            for i, t9 in enumerate((2, 3, 5, 6, 8)):
                tm = tmpp.tile([P, fd], f16, name=f"tm{y0}_{t9}", tag=f"tmd{i % 2}")
                nc.vector.tensor_scalar_mul(tm[:], view(t9), sc(t9))
                nc.vector.tensor_tensor(acc[:, :fd], tm[:], acc[:, :fd], op=ALU.add)
            for t9 in ACT_TAPS:
                nc.vector.tensor_tensor(
                    acc[:, :fd], tmps[t9][:], acc[:, :fd], op=ALU.add
                )
            return acc.rearrange("p (r w) -> p r w", w=IW)

        slabs = {}

        def mid1(hs):
            if hs in PE1:
                return None  # comes from PE mid copies
            slab = slabs[hs // 2]
            return slab[:, RS * (hs % 2) : RS * (hs % 2) + RS, :W]

        m0, m1 = {}, {}

        # ---- PE depthwise in slice pairs sharing each tap's stationary
        def dw_pair(cc, hs_a, hs_b=None):
            mts = []
            for hs in (hs_a, hs_b):
                if hs is None:
                    continue
                mts.append((hs, midps.tile([P, NS], f32, name="mid", tag="mid")))
            for t9 in range(K9):
                di, dj = t9 // 3, t9 % 3
                for hs, mt in mts:
                    nc.tensor.matmul(
                        mt[:],
                        dia[:, cc, t9, :],
                        imgv[cc][:, RS * hs + di : RS * hs + di + RS, dj : dj + W],
                        start=(t9 == 0),
                        stop=(t9 == 8),
                    )
            for hs, mt in mts:
                m = midsb.tile([P, NS], f16, name="m", tag="m")
                nc.scalar.copy(m[:], mt[:])
                (m0 if cc == 0 else m1)[hs] = m

        outv = out_d.rearrange("(o p) hw -> p o hw", o=CC)

        # out-copy engine rotation: ACT carries slab assists, DVE is busy
        # with slabs until ~2/3 in, gpsimd is idle -> early slices lean on
        # ACT+gpsimd, late slices on DVE.
        def out_copy(ob, oc, o_ps, eng):
            if eng == "v":
                nc.vector.tensor_copy(ob[:, oc, :], o_ps[:])
            else:
                nc.scalar.copy(ob[:, oc, :], o_ps[:])

        def one_by_one(hs, engs=("s", "s"), last=False):
            rhs0 = m0[hs][:]
            rhs1 = m1[hs][:] if hs in PE1 else mid1(hs)
            ob = outsb.tile([P, CC, NS], f16, name="ob", tag="ob")
            for oc in range(CC):
                o_ps = outps.tile([P, NS], f32, name="ops", tag="ops")
                nc.tensor.matmul(
                    o_ps[:], wct_v[:, 0, oc * P : (oc + 1) * P], rhs0,
                    start=True, stop=False,
                )
                nc.tensor.matmul(
                    o_ps[:], wct_v[:, 1, oc * P : (oc + 1) * P], rhs1,
                    start=False, stop=True,
                )
                out_copy(ob, oc, o_ps, engs[oc])
                if last:
                    dst = outv[:, :, hs * NS : (hs + 1) * NS]
                    eng = nc.sync if oc == 0 else nc.scalar
                    eng.dma_start(dst[:, oc : oc + 1, :], ob[:, oc : oc + 1, :])
            if not last:
                dst = outv[:, :, hs * NS : (hs + 1) * NS]
                nc.sync.dma_start(dst[:, :, :], ob[:, :, :])

        # ---- emission order (Tile scheduler refines by deps)
        slabs[0] = dve_slab(0, 16)
        dw_pair(0, 0, 1)
        dw_pair(0, 2, 3)
        one_by_one(0, engs=("s", "s"))
        one_by_one(1, engs=("s", "s"))
        slabs[1] = dve_slab(16, 16)
        dw_pair(0, 4, 5)
        one_by_one(2, engs=("s", "s"))
        one_by_one(3, engs=("s", "s"))
        slabs[2] = dve_slab(32, 8)
        dw_pair(0, 6, 7)
        dw_pair(1, PE1[0], PE1[1])
        dw_pair(1, PE1[2])
        one_by_one(4, engs=("v", "s"))
        one_by_one(5, engs=("s", "v"))
        one_by_one(6, engs=("v", "s"))
        one_by_one(7, engs=("v", "s"), last=True)

    nc.compile()
    return nc


def _get_nc():
    if "nc" not in _CACHE:
        _CACHE["nc"] = _build()
    return _CACHE["nc"]


def _prep_in_maps(image_feat, temp_feat, Wt, bt, Wf, bf, Wc, bc):
    f = lambda a: np.asarray(a, dtype=np.float64)
    image_feat = np.asarray(image_feat, dtype=np.float32)

    # ---- host MLP: per-sample unnormalized softmax weights
    t = f(temp_feat) @ f(Wt) + f(bt)                      # [bs, 32]
    logit = (t @ f(Wf) + f(bf)) / 100.0                   # [bs, c*9]
    e = np.exp(logit).reshape(BS, C, K9)                  # unnormalized
    r = 1.0 / e.sum(-1)                                   # [bs, c]

    img_pad = np.zeros((BS, C, IW, IW), np.float16)
    img_pad[:, :, 1 : H + 1, 1 : W + 1] = image_feat.astype(np.float16)
    img_pad = img_pad.reshape(BS, C, IMG)

    wct = f(Wc).T                                         # [c, o]

    in_maps = []
    for i in range(BS):
        blob = np.zeros((P, A_N), np.float32)
        wct_s = (wct * r[i][:, None]).astype(np.float16)  # [c, o]
        wct_p = np.ascontiguousarray(
            wct_s.reshape(CC, P, C).transpose(1, 0, 2).reshape(P, CC * C)
        )
        blob[:, A_WCT : A_WCT + 256] = wct_p.view(np.float32)
        # e fp32 scalars [p, cc*9+t]
        ei = e[i].reshape(CC, P, K9).transpose(1, 0, 2).reshape(P, CC * K9)
        blob[:, A_E : A_E + CC * K9] = ei.astype(np.float32)
        # diag stationaries [p, cc, t, q] fp16: diag(e)
        dia = np.zeros((P, CC, K9, P), np.float16)
        idx = np.arange(P)
        for cc in range(CC):
            for t9 in range(K9):
                dia[idx, cc, t9, idx] = e[i, cc * P : (cc + 1) * P, t9].astype(
                    np.float16
                )
        in_maps.append(
            {
                "img": img_pad[i],
                "bla": blob,
                "dia": dia.reshape(P, CC * K9 * P),
            }
        )
    return in_maps


def kernel(image_feat, temp_feat, Wt, bt, Wf, bf, Wc, bc):
    from concourse.bass_utils import run_bass_kernel_spmd

    nc = _get_nc()
    in_maps = _prep_in_maps(image_feat, temp_feat, Wt, bt, Wf, bf, Wc, bc)
    res = run_bass_kernel_spmd(nc, in_maps, core_ids=list(range(BS)))
    _CACHE["last_result"] = res
    out = np.stack([res.results[i]["out"] for i in range(BS)], axis=0)
    out = out.reshape(BS, C, H, W).astype(np.float32)
    out += np.asarray(bc, dtype=np.float32)[None, :, None, None]
    return out


# revision 20
# speedup vs baseline: 1.3429x; 1.0102x over previous
"""AdjustableConvolution2d Trainium2 kernel (v6).

Data-parallel over batch: 8 samples -> 8 NeuronCores, no collectives.

The per-sample filter MLP (temp@Wt -> @Wf -> softmax) is tiny, so it runs
on the HOST in fp64/fp32; the device receives finished artifacts:
  - diag: diag(e) stationaries [128, cc*9*128] fp16 (e = unnormalized
    exp(logits), ~1 +- 2%),
  - e32:  the same e as fp32 per-partition scalars for the DVE/ACT taps,
  - wct_s: Wc.T pre-scaled by the softmax denominator 1/sum(e) fp16.
Bias bc is added on the host after the kernel (free), so the device
pipeline is pure dw + 1x1:

  PE (TensorEngine): keepers to warm the HAM clock-gate, then depthwise
    diag-matmuls for cc0 slices 0-7 and cc1 slices 6-7 in slice pairs
    sharing each tap's stationary, interleaved with the 1x1
    channel-combine (2 matmuls/psum per (slice, oc)).
  DVE: cc1 slices 0-5 as three 2-slice slabs in the 66-wide junk-column
    layout; each tap is tensor_scalar (2x mode, ~0.47ns/elem) into a tmp
    then tensor_tensor add (~0.67ns/elem) -- measured 1.9x cheaper than
    the 1x scalar_tensor_tensor fused form.  Some tap scale-steps go to
    the ACT engine (activation scale=e ptr) and some adds to gpsimd to
    keep the DVE off the critical path.
  ACT: PSUM->SBUF mid copies for PE dw units, slab scale-assists, and a
    share of the output copies.  GpSimd: slab add-assists + out copies.
Output fp16 (no bias), upcast + bias on host.
"""

import sys

if "/opt/trn_rl_repo" not in sys.path:
    sys.path.insert(0, "/opt/trn_rl_repo")

import numpy as np

BS, C, H, W = 8, 256, 64, 64
P = 128
CC = C // P            # 2 channel chunks of 128
IW = W + 2             # 66: padded row width (junk-column layout)
IMG = IW * IW          # 4356
RS = 8                 # output rows per slice
NS = RS * W            # 512 columns per slice
NSL = H // RS          # 8 slices
K9 = 9

# blob fp32 column layout (128 partitions)
A_WCT = 0              # wct_s fp16 pairs [p, cc*256+o] -> 256 f32 cols
A_E = 256              # e fp32 [p, cc*9+t] -> 18 cols
A_N = 274

KEEPERS = 135          # junk matmuls warming the PE HAM clock gate
PE1 = (6, 7)           # cc1 slices computed on the PE (rest on DVE slabs)

_CACHE = {}


def _build():
    from contextlib import ExitStack

    import concourse.bass as bass
    import concourse.bacc as bacc
    import concourse.mybir as mybir
    import concourse.tile as tile

    dt = mybir.dt
    f32 = dt.float32
    f16 = dt.float16
    AF = mybir.ActivationFunctionType
    ALU = mybir.AluOpType

    nc = bacc.Bacc(
        "TRN2", target_bir_lowering=False, debug=False, enable_asserts=False
    )

    img_d = nc.dram_tensor("img", [C, IMG], f16, kind="ExternalInput")
    bla_d = nc.dram_tensor("bla", [P, A_N], f32, kind="ExternalInput")
    dia_d = nc.dram_tensor("dia", [P, CC * K9 * P], f16, kind="ExternalInput")
    out_d = nc.dram_tensor("out", [C, H * W], f16, kind="ExternalOutput")

    with tile.TileContext(nc) as tc, ExitStack() as ctx:
        constp = ctx.enter_context(tc.tile_pool(name="const", bufs=1))
        imgp = ctx.enter_context(tc.tile_pool(name="img", bufs=1))
        accp = ctx.enter_context(tc.tile_pool(name="accp", bufs=1))
        tmpp = ctx.enter_context(tc.tile_pool(name="tmpp", bufs=1))
        midsb = ctx.enter_context(tc.tile_pool(name="midsb", bufs=12))
        outsb = ctx.enter_context(tc.tile_pool(name="outsb", bufs=6))
        midps = ctx.enter_context(
            tc.tile_pool(name="midps", bufs=3, space=bass.MemorySpace.PSUM)
        )
        outps = ctx.enter_context(
            tc.tile_pool(name="outps", bufs=3, space=bass.MemorySpace.PSUM)
        )

        # tiny fp16 zero tile for PE keepers, available immediately
        kzero = constp.tile([P, 32], f16)
        nc.gpsimd.memset(kzero[:], 0.0)

        # ---- input DMAs, priority order: the PE's first dw pair needs dia
        # cc0 + img cc0 rows 0..9; the first DVE slab needs e32 (bla) + img
        # cc1 rows 0..17.  ~3MB of input is bandwidth-bound (~8us), so the
        # first-needed pieces go first across both queues.
        bla = constp.tile([P, A_N], f32)
        dia = constp.tile([P, CC, K9, P], f16)
        dia_v = dia_d.rearrange("p (cc t q) -> p cc t q", cc=CC, t=K9)
        img_sb = imgp.tile([P, CC, IMG], f16)

        # all input DMAs ride the sync queue (the scalar/ACT queue must stay
        # clear for slab assists + copies); priority order by first need.
        nc.sync.dma_start(dia[:, 0], dia_v[:, 0])
        nc.sync.dma_start(img_sb[:, 0, : 10 * IW], img_d[0:P, : 10 * IW])
        nc.sync.dma_start(img_sb[:, 1, : 18 * IW], img_d[P : 2 * P, : 18 * IW])
        nc.sync.dma_start(bla[:], bla_d[:])
        nc.sync.dma_start(
            img_sb[:, 0, 10 * IW : 18 * IW], img_d[0:P, 10 * IW : 18 * IW]
        )
        nc.sync.dma_start(dia[:, 1], dia_v[:, 1])
        splits = ((18 * IW, 36 * IW), (36 * IW, 52 * IW), (52 * IW, IMG))
        for lo, hi in splits:
            nc.sync.dma_start(img_sb[:, 0, lo:hi], img_d[0:P, lo:hi])
            nc.sync.dma_start(img_sb[:, 1, lo:hi], img_d[P : 2 * P, lo:hi])

        wct_v = bla[:, A_WCT : A_WCT + 256].bitcast(f16).rearrange(
            "p (cc o) -> p cc o", cc=CC
        )
        e32 = bla[:, A_E : A_E + CC * K9].rearrange("p (cc t) -> p cc t", cc=CC)

        # ---- PE keepers: warm the HAM clock gate while DMAs land
        k_ps = midps.tile([P, NS], f32, name="mid", tag="mid")
        for _ in range(KEEPERS):
            nc.tensor.matmul(k_ps[:1, :32], kzero[:, :1], kzero[:], skip_group_check=True)

        imgv = [
            img_sb[:, cc, :].rearrange("p (r w) -> p r w", w=IW) for cc in range(CC)
        ]
        img1 = img_sb[:, 1, :]

        # ---- DVE slabs: cc1 slices 0-4 as two 2-slice slabs + one single.
        # Per tap: tmp = img_view * e (TS on DVE at ~2x, or ACT activation),
        # then acc += tmp (TT on DVE at ~2x).  No gpsimd: its tensor ops
        # lock the DVE's shared SBUF port and throttle concurrent DVE work
        # ~5x (measured).  Chain order: own-scaled taps first, ACT-fed adds
        # last, so the cross-engine tmps have time to land.
        ACT_TAPS = (1, 4, 7)       # scale-step on ACT

        def dve_slab(y0, nrows):
            fd = (nrows - 1) * IW + W
            acc = accp.tile([P, nrows * IW], f16, name=f"acc{y0}", tag=f"acc{y0}")
            sc = lambda t: e32[:, 1, t : t + 1]

            def view(t9):
                di, dj = t9 // 3, t9 % 3
                s0 = (y0 + di) * IW + dj
                return img1[:, s0 : s0 + fd]

            tmps = {}
            for t9 in ACT_TAPS:
                tm = tmpp.tile([P, fd], f16, name=f"tm{y0}_{t9}", tag=f"tmp{t9}")
                nc.scalar.activation(tm[:], view(t9), AF.Identity, scale=sc(t9))
                tmps[t9] = tm
            # DVE chain: own taps (TS+TT) first, ACT-fed adds last
            nc.vector.tensor_scalar_mul(acc[:, :fd], view(0), sc(0))
            for i, t9 in enumerate((2, 3, 5, 6, 8)):
                tm = tmpp.tile([P, fd], f16, name=f"tm{y0}_{t9}", tag=f"tmd{i % 2}")
                nc.vector.tensor_scalar_mul(tm[:], view(t9), sc(t9))
                nc.vector.tensor_tensor(acc[:, :fd], tm[:], acc[:, :fd], op=ALU.add)
            for t9 in ACT_TAPS:
                nc.vector.tensor_tensor(
                    acc[:, :fd], tmps[t9][:], acc[:, :fd], op=ALU.add
                )
            return acc.rearrange("p (r w) -> p r w", w=IW)

        slabs = {}

        def mid1(hs):
            if hs in PE1:
                return None  # comes from PE mid copies
            slab = slabs[hs // 2]
            return slab[:, RS * (hs % 2) : RS * (hs % 2) + RS, :W]

        m0, m1 = {}, {}

        # ---- PE depthwise in slice pairs sharing each tap's stationary
        def dw_pair(cc, hs_a, hs_b=None, cp="s"):
            mts = []
            for hs in (hs_a, hs_b):
                if hs is None:
                    continue
                mts.append((hs, midps.tile([P, NS], f32, name="mid", tag="mid")))
            for t9 in range(K9):
                di, dj = t9 // 3, t9 % 3
                for hs, mt in mts:
                    nc.tensor.matmul(
                        mt[:],
                        dia[:, cc, t9, :],
                        imgv[cc][:, RS * hs + di : RS * hs + di + RS, dj : dj + W],
                        start=(t9 == 0),
                        stop=(t9 == 8),
                    )
            for hs, mt in mts:
                m = midsb.tile([P, NS], f16, name="m", tag="m")
                if cp == "v":
                    nc.vector.tensor_copy(m[:], mt[:])
                else:
                    nc.scalar.copy(m[:], mt[:])
                (m0 if cc == 0 else m1)[hs] = m

        outv = out_d.rearrange("(o p) hw -> p o hw", o=CC)

        # out-copy engine rotation: ACT carries slab assists, DVE is busy
        # with slabs until ~2/3 in, gpsimd is idle -> early slices lean on
        # ACT+gpsimd, late slices on DVE.
        def out_copy(ob, oc, o_ps, eng):
            if eng == "v":
                nc.vector.tensor_copy(ob[:, oc, :], o_ps[:])
            else:
                nc.scalar.copy(ob[:, oc, :], o_ps[:])

        def one_by_one(hs, engs=("s", "s"), last=False):
            rhs0 = m0[hs][:]
            rhs1 = m1[hs][:] if hs in PE1 else mid1(hs)
            ob = outsb.tile([P, CC, NS], f16, name="ob", tag="ob")
            for oc in range(CC):
                o_ps = outps.tile([P, NS], f32, name="ops", tag="ops")
                nc.tensor.matmul(
                    o_ps[:], wct_v[:, 0, oc * P : (oc + 1) * P], rhs0,
                    start=True, stop=False,
                )
                nc.tensor.matmul(
                    o_ps[:], wct_v[:, 1, oc * P : (oc + 1) * P], rhs1,
                    start=False, stop=True,
                )
                out_copy(ob, oc, o_ps, engs[oc])
                if last:
                    dst = outv[:, :, hs * NS : (hs + 1) * NS]
                    eng = nc.sync if oc == 0 else nc.scalar
                    eng.dma_start(dst[:, oc : oc + 1, :], ob[:, oc : oc + 1, :])
            if not last:
                dst = outv[:, :, hs * NS : (hs + 1) * NS]
                nc.sync.dma_start(dst[:, :, :], ob[:, :, :])

        # ---- emission order (Tile scheduler refines by deps)
        slabs[0] = dve_slab(0, 16)
        dw_pair(0, 0, 1)
        dw_pair(0, 2, 3)
        one_by_one(0, engs=("s", "s"))
        one_by_one(1, engs=("s", "s"))
        slabs[1] = dve_slab(16, 16)
        dw_pair(0, 4, 5)
        one_by_one(2, engs=("s", "s"))
        one_by_one(3, engs=("s", "s"))
        slabs[2] = dve_slab(32, 16)
        dw_pair(0, 6, 7)
        dw_pair(1, PE1[0], PE1[1], cp="v")
        one_by_one(4, engs=("v", "s"))
        one_by_one(5, engs=("s", "v"))
        one_by_one(6, engs=("v", "s"))
        one_by_one(7, engs=("v", "s"), last=True)

    nc.compile()
    return nc


def _get_nc():
    if "nc" not in _CACHE:
        _CACHE["nc"] = _build()
    return _CACHE["nc"]


def _prep_in_maps(image_feat, temp_feat, Wt, bt, Wf, bf, Wc, bc):
    f = lambda a: np.asarray(a, dtype=np.float64)
    image_feat = np.asarray(image_feat, dtype=np.float32)

    # ---- host MLP: per-sample unnormalized softmax weights
    t = f(temp_feat) @ f(Wt) + f(bt)                      # [bs, 32]
    logit = (t @ f(Wf) + f(bf)) / 100.0                   # [bs, c*9]
    e = np.exp(logit).reshape(BS, C, K9)                  # unnormalized
    r = 1.0 / e.sum(-1)                                   # [bs, c]

    img_pad = np.zeros((BS, C, IW, IW), np.float16)
    img_pad[:, :, 1 : H + 1, 1 : W + 1] = image_feat.astype(np.float16)
    img_pad = img_pad.reshape(BS, C, IMG)

    wct = f(Wc).T                                         # [c, o]

    in_maps = []
    for i in range(BS):
        blob = np.zeros((P, A_N), np.float32)
        wct_s = (wct * r[i][:, None]).astype(np.float16)  # [c, o]
        wct_p = np.ascontiguousarray(
            wct_s.reshape(CC, P, C).transpose(1, 0, 2).reshape(P, CC * C)
        )
        blob[:, A_WCT : A_WCT + 256] = wct_p.view(np.float32)
        # e fp32 scalars [p, cc*9+t]
        ei = e[i].reshape(CC, P, K9).transpose(1, 0, 2).reshape(P, CC * K9)
        blob[:, A_E : A_E + CC * K9] = ei.astype(np.float32)
        # diag stationaries [p, cc, t, q] fp16: diag(e)
        dia = np.zeros((P, CC, K9, P), np.float16)
        idx = np.arange(P)
        for cc in range(CC):
            for t9 in range(K9):
                dia[idx, cc, t9, idx] = e[i, cc * P : (cc + 1) * P, t9].astype(
                    np.float16
                )
        in_maps.append(
            {
                "img": img_pad[i],
                "bla": blob,
                "dia": dia.reshape(P, CC * K9 * P),
            }
        )
    return in_maps


def kernel(image_feat, temp_feat, Wt, bt, Wf, bf, Wc, bc):
    from concourse.bass_utils import run_bass_kernel_spmd

    nc = _get_nc()
    in_maps = _prep_in_maps(image_feat, temp_feat, Wt, bt, Wf, bf, Wc, bc)
    res = run_bass_kernel_spmd(nc, in_maps, core_ids=list(range(BS)))
    _CACHE["last_result"] = res
    out = np.stack([res.results[i]["out"] for i in range(BS)], axis=0)
    out = out.reshape(BS, C, H, W).astype(np.float32)
    out += np.asarray(bc, dtype=np.float32)[None, :, None, None]
    return out
